# revision 38
# baseline (speedup 1.0000x reference)
"""Trainium2 Bass kernel for relative-position multi-head attention
(Transformer-XL style), sharded over 8 NeuronCores by head (2 heads/core)
with row-parallel output projection (partial sums reduced on host).

Math (per core c, d-slice = rows 128c..128c+128 of the projection space):
  qT = Wq[ds] @ Q.T          (128, L)   [+ bq]
  kT = Wk[ds] @ K.T + bk     (128, L)
  v  = V @ Wv[ds].T          (L, 128)   [bv folded on host]
  per head h (64-row slice of the 128):
    S  = ((q+u)/8).T @ k  +  shift(((q+v)/8).T @ F.T)   (L, L)
    P  = exp(S);  out = (P @ v_h) / P.sum(1)
  O_part = concat(out).T-contraction with Wo[:, ds]  ->  O^T (1024, L) f32
Host: out = (sum_c O_part).T + bo + bv @ Wo.T

Structure per 128-row l-tile:
  - AC psum chunks (4x512) stay in PSUM.
  - band matmul (width 2176) vs F = flip(rel_emb) in 512-col psum chunks,
    copied to SBUF bf16.
  - diagonal gather DMA (HWDGE on SP): s_s[p, j] = band[p, 127-p+j]
    (plain write, no accum — RMW/SWDGE was the old bottleneck).
  - DVE/Pool adds: s2 = AC_psum + s_s;  Act: p_s = exp(s2).
  - one batched XBAR transpose per l-tile (Act queue):
    ptall[p, mt*512 + lt4*128 + l] = p_s[l, mt*128+p].
  - PV accumulates over m-tiles (ones column gives Z), normalize, Wo
    matmuls, and direct PSUM->DRAM output stores.
"""

import math
import numpy as np
import ml_dtypes

import concourse.bass as bass
import concourse.bacc as bacc
import concourse.mybir as mybir
import concourse.tile as tile
from concourse.bass_utils import run_bass_kernel_spmd
from contextlib import ExitStack

BF16 = mybir.dt.bfloat16
F32 = mybir.dt.float32
AF = mybir.ActivationFunctionType
ALU = mybir.AluOpType

L = 2048          # sequence length
D = 1024          # model dim
DK = 64           # head dim
NH = 16           # total heads
NCORES = 8
DH = 128          # per-core projection slice (2 heads * 64)
LT = 128          # l-tile rows
NLT = L // LT     # 16 l-tiles
MC = 512          # m-chunk for AC matmuls
BW = 2176         # band width per l-tile (2175 rounded up to 17*128)
LC = 512          # l-chunk for PV/Wo stage
NLC = L // LC     # 4


_QCFG = {"diag": "sync", "diag2": "scalar", "trans": "sync",
         "trans2": "scalar", "vx": "gpsimd", "out": "gpsimd",
         "qx": "gpsimd", "kx": "scalar"}


def _build_module(ablate=frozenset(), qcfg=None, sc_bufs=3, ac_bufs=2, bd_bufs=2, depth=2, pt_bufs=None):
    qc = dict(_QCFG)
    if qcfg:
        qc.update(qcfg)
    nc = bacc.Bacc("TRN2", target_bir_lowering=False, debug=False,
                   enable_asserts=False, num_devices=NCORES)
    qeng = {k: getattr(nc, v) for k, v in qc.items()}

    # ---- DRAM I/O ----
    d_qt = nc.dram_tensor("qt", (D, L), BF16, kind="ExternalInput")
    d_kt = nc.dram_tensor("kt", (D, L), BF16, kind="ExternalInput")
    d_vt = nc.dram_tensor("vt", (D, L), BF16, kind="ExternalInput")
    d_wqt = nc.dram_tensor("wqt", (D, DH), BF16, kind="ExternalInput")
    d_wkt = nc.dram_tensor("wkt", (D, DH), BF16, kind="ExternalInput")
    d_wvt = nc.dram_tensor("wvt", (D, DH), BF16, kind="ExternalInput")
    d_wot = nc.dram_tensor("wot", (DH, D), BF16, kind="ExternalInput")
    d_ft = nc.dram_tensor("ft", (DK, 4096), BF16, kind="ExternalInput")
    d_ub = nc.dram_tensor("ubias", (DH, 1), F32, kind="ExternalInput")
    d_vb = nc.dram_tensor("vbias", (DH, 1), F32, kind="ExternalInput")
    d_kb = nc.dram_tensor("kbias", (DH, 1), F32, kind="ExternalInput")
    d_ones2 = nc.dram_tensor("ones2", (2, DK), F32, kind="ExternalInput")
    d_out = nc.dram_tensor("opart", (D, L), BF16, kind="ExternalOutput")

    with tile.TileContext(nc) as tc, ExitStack() as ctx:
        const = ctx.enter_context(tc.tile_pool(name="const", bufs=1))
        persist = ctx.enter_context(tc.tile_pool(name="persist", bufs=1))

        # ---- persistent SBUF loads ----
        ft_s = const.tile([128, 4096], BF16)
        nc.sync.dma_start(ft_s[DK:128, :], d_ft[:])
        wqt_s = const.tile([128, 8 * DH], BF16)
        wkt_s = const.tile([128, 8 * DH], BF16)
        wvt_s = const.tile([128, 8 * DH], BF16)
        for dd, ss in ((d_wqt, wqt_s), (d_wkt, wkt_s), (d_wvt, wvt_s)):
            nc.sync.dma_start(
                ss[:].rearrange("p (a d) -> p a d", a=8),
                dd[:].rearrange("(a p) d -> p a d", p=128))
        wot_s = const.tile([DH, D], BF16)
        nc.sync.dma_start(wot_s[:], d_wot[:])
        ub_s = const.tile([DH, 1], F32)
        vb_s = const.tile([DH, 1], F32)
        kb_s = const.tile([DH, 1], F32)
        nc.sync.dma_start(ub_s[:], d_ub[:])
        nc.sync.dma_start(vb_s[:], d_vb[:])
        nc.sync.dma_start(kb_s[:], d_kb[:])

        # per-head projection outputs (64-row slices)
        quv = [persist.tile([128, L], BF16, tag=f"quv{h}", name=f"quv{h}") for h in range(2)]
        kth = [persist.tile([DK, L], BF16, tag=f"kth{h}", name=f"kth{h}") for h in range(2)]
        # v in m-major layout: vall[p, mt*128 + d] = v[m = mt*128+p, d]
        vall = persist.tile([128, L], BF16, tag="vall", name="vall")

        # ---- q/k projections ----
        with tc.tile_pool(name="xin", bufs=1) as xin, \
             tc.tile_pool(name="prjp", bufs=2, space="PSUM") as prjp:
            qx = xin.tile([128, 8 * L], BF16, tag="qx", name="qx")
            kx = xin.tile([128, 8 * L], BF16, tag="kx", name="kx")
            qeng["qx"].dma_start(
                qx[:].rearrange("p (a l) -> p a l", a=8),
                d_qt[:].rearrange("(a p) l -> p a l", p=128))
            qeng["kx"].dma_start(
                kx[:].rearrange("p (a l) -> p a l", a=8),
                d_kt[:].rearrange("(a p) l -> p a l", p=128))
            for lc4 in range(4):
                qp = prjp.tile([128, 512], F32, tag="qp")
                kp = prjp.tile([128, 512], F32, tag="kp")
                for cc in range(8):
                    wsl = bass.ts(cc, DH)
                    xsl = slice(cc * L + lc4 * 512, cc * L + (lc4 + 1) * 512)
                    nc.tensor.matmul(qp[:], wqt_s[:, wsl], qx[:, xsl],
                                     start=(cc == 0), stop=(cc == 7))
                    nc.tensor.matmul(kp[:], wkt_s[:, wsl], kx[:, xsl],
                                     start=(cc == 0), stop=(cc == 7))
                lsl = bass.ts(lc4, 512)
                for h in range(2):
                    hsl = slice(h * DK, (h + 1) * DK)
                    nc.scalar.activation(quv[h][0:DK, lsl], qp[hsl, :], AF.Identity,
                                         bias=ub_s[hsl, :], scale=0.125)
                    nc.scalar.activation(quv[h][DK:128, lsl], qp[hsl, :], AF.Identity,
                                         bias=vb_s[hsl, :], scale=0.125)
                    nc.scalar.activation(kth[h][:, lsl], kp[hsl, :], AF.Identity,
                                         bias=kb_s[hsl, :])

        # ---- v projection: dh-major matmuls, then one XBAR transpose ----
        with tc.tile_pool(name="vin", bufs=1) as vin, \
             tc.tile_pool(name="vp", bufs=2, space="PSUM") as vp:
            vx = vin.tile([128, 8 * L], BF16, tag="vx", name="vx")
            qeng["vx"].dma_start(
                vx[:].rearrange("p (a l) -> p a l", a=8),
                d_vt[:].rearrange("(a p) l -> p a l", p=128))
            vts = vin.tile([DH, L], BF16, tag="vts", name="vts")
            for lc4 in range(4):
                pv = vp.tile([128, 512], F32, tag="pv")
                for cc in range(8):
                    xsl = slice(cc * L + lc4 * 512, cc * L + (lc4 + 1) * 512)
                    nc.tensor.matmul(pv[:], wvt_s[:, bass.ts(cc, DH)],
                                     vx[:, xsl],
                                     start=(cc == 0), stop=(cc == 7))
                if lc4 % 2 == 0:
                    nc.scalar.activation(vts[:, bass.ts(lc4, 512)], pv[:],
                                         AF.Copy)
                else:
                    nc.vector.tensor_copy(vts[:, bass.ts(lc4, 512)], pv[:])
            # vall[p, mt, d] = vts[d, mt*128+p]
            vto = bass.AP(vall[:].tensor, 0,
                          [[L, 128], [128, NLT], [1, 128]])
            nc.sync.dma_start_transpose(vto, vts[:])

        # ---- attention + output ----
        ones2 = const.tile([2, DK], F32)
        nc.sync.dma_start(ones2[:], d_ones2[:])
        with tc.tile_pool(name="sc", bufs=sc_bufs) as sc, \
             tc.tile_pool(name="pt", bufs=pt_bufs or depth + 1) as ptp, \
             tc.tile_pool(name="at", bufs=2) as atp, \
             tc.tile_pool(name="acp", bufs=ac_bufs, space="PSUM") as acp, \
             tc.tile_pool(name="bdp", bufs=bd_bufs, space="PSUM") as bdp, \
             tc.tile_pool(name="pvp", bufs=1, space="PSUM") as pvp, \
             tc.tile_pool(name="wop", bufs=1, space="PSUM") as wop, \
             tc.tile_pool(name="oev", bufs=2) as oev:
            def emit_s(lc, h, ptall):
                for lt4 in range(NLC):
                    ltg = lc * 4 + lt4
                    l0 = ltg * LT
                    b0 = 1920 - l0
                    lhq = quv[h][0:DK, l0:l0 + LT]
                    lhv = quv[h][DK:128, l0:l0 + LT]
                    s_s = sc.tile([LT, L], BF16, tag="s")
                    ac_s = sc.tile([LT, L], BF16, tag="ac_s")
                    band = sc.tile([LT, BW], BF16, tag="band")
                    # AC -> psum -> sbuf immediately (frees psum depth)
                    aps = []
                    for q2 in range(2):
                        ap = acp.tile([LT, 1024], F32, tag="ac")
                        for qq in range(2 if "ac" not in ablate else 0):
                            nc.tensor.matmul(
                                ap[:, bass.ts(qq, MC)], lhq,
                                kth[h][:, bass.ts(q2 * 2 + qq, MC)],
                                start=True, stop=True)
                        if "ac" in ablate:
                            nc.vector.memset(ap[:], 0.0)
                        if "psumadd" not in ablate and not (
                                "nohh" not in ablate and q2 == 0):
                            nc.vector.tensor_copy(
                                ac_s[:, bass.ts(q2, 1024)], ap[:])
                        aps.append(ap)
                    # band: 4x512 chunks + 128 tail, 1-bank psums
                    for q5 in range(4 if "band" not in ablate else 0):
                        bp = bdp.tile([LT, MC], F32, tag="bd")
                        nc.tensor.matmul(bp[:], lhv,
                                         ft_s[DK:128,
                                              b0 + q5 * MC:b0 + (q5 + 1) * MC],
                                         start=True, stop=True)
                        dst = band[:, bass.ts(q5, MC)]
                        if q5 % 2 == 0:
                            nc.scalar.activation(dst, bp[:], AF.Copy)
                        else:
                            nc.vector.tensor_copy(dst, bp[:])
                    if "band" not in ablate:
                        bt = bdp.tile([LT, 128], F32, tag="bd", name="bt")
                        nc.tensor.matmul(bt[:], lhv,
                                         ft_s[DK:128, b0 + 2048:b0 + BW],
                                         start=True, stop=True)
                        nc.scalar.activation(band[:, 2048:BW], bt[:],
                                             AF.Copy)
                    if "diag" not in ablate and "band" not in ablate:
                        # diagonal gather: s_s[p, j] = band[p, 127-p+j]
                        diag = bass.AP(band[:].tensor, 127,
                                       [[BW - 1, LT], [1, L]])
                        if "diag3" in qc and qc["diag3"]:
                            dl = [qeng["diag"], qeng["diag2"],
                                  qeng["diag3"]]
                            deng = dl[ltg % 3]
                        else:
                            deng = (qeng.get("diag2") if lt4 % 2 == 0
                                    else qeng["diag"]) or qeng["diag"]
                        deng.dma_start(s_s[:], diag)
                    else:
                        nc.gpsimd.memset(s_s[:], 0.0)
                    if "adds" not in ablate:
                        if "nohh" not in ablate:
                            nc.vector.tensor_tensor(
                                s_s[:, 0:1024], aps[0][:],
                                s_s[:, 0:1024], ALU.add)
                            nc.gpsimd.tensor_tensor(
                                s_s[:, 1024:2048], ac_s[:, 1024:2048],
                                s_s[:, 1024:2048], ALU.add)
                        elif "psumadd" not in ablate and "gpadd" not in ablate:
                            # sbuf-bf16 adds: DVE (2x mode) + idle Pool
                            nc.vector.tensor_tensor(
                                s_s[:, 0:1024], ac_s[:, 0:1024],
                                s_s[:, 0:1024], ALU.add)
                            nc.gpsimd.tensor_tensor(
                                s_s[:, 1024:2048], ac_s[:, 1024:2048],
                                s_s[:, 1024:2048], ALU.add)
                        elif "psumadd" in ablate:
                            for q2 in range(2):
                                nc.vector.tensor_tensor(
                                    s_s[:, bass.ts(q2, 1024)], aps[q2][:],
                                    s_s[:, bass.ts(q2, 1024)], ALU.add)
                        else:
                            nc.gpsimd.tensor_tensor(s_s[:], ac_s[:],
                                                    s_s[:], ALU.add)
                    # exp with row-sum (Z) accumulated on the side;
                    # Z rides the transpose as column 2048 of p_s
                    p_s = sc.tile([LT, BW], BF16, tag="p")
                    zacc = oev.tile([LT, 1], F32, tag="zacc")
                    nc.scalar.activation(p_s[:, 0:L], s_s[:],
                                         AF.Exp if "exp" not in ablate
                                         else AF.Copy,
                                         accum_out=zacc[:])
                    nc.gpsimd.tensor_copy(p_s[:, L:L + 1], zacc[:])
                    nc.gpsimd.memset(p_s[:, L + 1:BW], 0.0)
                    if "trans" not in ablate:
                        # batched XBAR transpose:
                        # out[p, mt, l] = p_s[l, mt*128+p]
                        tout = bass.AP(ptall[:].tensor, lt4 * LT,
                                       [[(NLT + 1) * LC, 128],
                                        [LC, NLT + 1], [1, LT]])
                        teng = (qeng.get("trans2") if ltg % 2
                                else qeng["trans"]) or qeng["trans"]
                        teng.dma_start_transpose(tout, p_s[:])
                    elif lt4 == 0:
                        nc.gpsimd.memset(ptall[:], 0.0)

            def emit_pv(lc, h, ptall, at_s):
                if "pv" in ablate:
                    nc.gpsimd.memset(at_s[h * DK:(h + 1) * DK, :], 0.5)
                    return
                # PV: accumulate over m-tiles
                po = pvp.tile([DK, LC], F32, tag="po")
                for mt in range(NLT):
                    nc.tensor.matmul(
                        po[:],
                        vall[:, mt * 128 + h * DK:mt * 128 + (h + 1) * DK],
                        ptall[:, bass.ts(mt, LC)],
                        start=(mt == 0), stop=(mt == NLT - 1))
                # normalize: recipZ broadcast via K=2 ones-matmul;
                # Z sits in ptall block 16, row 0
                rz = oev.tile([2, LC], F32, tag="rz")
                nc.gpsimd.memset(rz[:], 0.0)
                nc.vector.reciprocal(rz[0:1, :],
                                     ptall[0:1, bass.ts(NLT, LC)])
                bc = bdp.tile([DK, LC], F32, tag="bd", name="bc")
                nc.tensor.matmul(bc[:], ones2[:], rz[:],
                                 start=True, stop=True)
                bcs = oev.tile([DK, LC], F32, tag="bcs")
                nc.scalar.activation(bcs[:], bc[:], AF.Copy)
                nc.vector.tensor_tensor(at_s[h * DK:(h + 1) * DK, :],
                                        po[:], bcs[:],
                                        ALU.mult)

            def emit_wo(lc, at_s):
                # Wo: 8 e-tiles; bf16 partials, one batched store per lc
                osb = oev.tile([128, 8 * LC], BF16, tag="osb")
                for et in range(8):
                    wp = wop.tile([128, LC], F32, tag="wo")
                    nc.tensor.matmul(wp[:], wot_s[:, bass.ts(et, 128)],
                                     at_s[:], start=True, stop=True)
                    if et % 2 == 0:
                        nc.scalar.activation(osb[:, bass.ts(et, LC)], wp[:],
                                             AF.Copy)
                    else:
                        nc.vector.tensor_copy(osb[:, bass.ts(et, LC)], wp[:])
                # d_out[(et*128+p), lc*512+l] = osb[p, et*512+l]
                dsto = bass.AP(d_out[:].tensor, lc * LC,
                               [[L, 128], [128 * L, 8], [1, LC]])
                qeng["out"].dma_start(dsto, osb[:].rearrange(
                    "p (a l) -> p a l", a=8))

            # software-pipelined emission: S-stage runs DEPTH groups ahead
            # of PV/Wo so PE always has ready work
            DEPTH = depth
            at_tiles = {}
            pend = []
            for lc in range(NLC):
                for h in range(2):
                    if h == 0:
                        at_tiles[lc] = atp.tile([DH, LC], BF16, tag="at",
                                                name=f"at{lc}")
                    ptall = ptp.tile([128, (NLT + 1) * LC], BF16, tag="pt",
                                     name="ptall")
                    emit_s(lc, h, ptall)
                    pend.append((lc, h, ptall))
                    if len(pend) > DEPTH:
                        plc, ph, ppt = pend.pop(0)
                        emit_pv(plc, ph, ppt, at_tiles[plc])
                        if ph == 1:
                            emit_wo(plc, at_tiles[plc])
            for plc, ph, ppt in pend:
                emit_pv(plc, ph, ppt, at_tiles[plc])
                if ph == 1:
                    emit_wo(plc, at_tiles[plc])
    nc.compile()
    return nc


_MODULE_CACHE = {}


def _get_module():
    if "nc" not in _MODULE_CACHE:
        _MODULE_CACHE["nc"] = _build_module()
    return _MODULE_CACHE["nc"]


def kernel(**inputs) -> np.ndarray:
    Q = np.asarray(inputs["Q"], np.float32)[0]      # (L, D)
    K = np.asarray(inputs["K"], np.float32)[0]
    V = np.asarray(inputs["V"], np.float32)[0]
    Wq = np.asarray(inputs["Wq"], np.float32)
    Wk = np.asarray(inputs["Wk"], np.float32)
    Wv = np.asarray(inputs["Wv"], np.float32)
    Wo = np.asarray(inputs["Wo"], np.float32)
    bq = np.asarray(inputs["bq"], np.float32)
    bk = np.asarray(inputs["bk"], np.float32)
    bv = np.asarray(inputs["bv"], np.float32)
    bo = np.asarray(inputs["bo"], np.float32)
    E = np.asarray(inputs["rel_emb"], np.float32)   # (4096, 64)
    u_b = np.asarray(inputs["u_bias"], np.float32)  # (16, 64)
    v_b = np.asarray(inputs["v_bias"], np.float32)

    bf = ml_dtypes.bfloat16
    QT = np.ascontiguousarray(Q.T).astype(bf)
    KT = np.ascontiguousarray(K.T).astype(bf)
    VT = np.ascontiguousarray(V.T).astype(bf)
    FT = np.ascontiguousarray(E[::-1].T).astype(bf)  # (64, 4096)

    in_maps = []
    for c in range(NCORES):
        ds = slice(DH * c, DH * c + DH)
        urep = np.concatenate([u_b[2 * c], u_b[2 * c + 1]])[:, None]
        vrep = np.concatenate([v_b[2 * c], v_b[2 * c + 1]])[:, None]
        in_maps.append({
            "qt": QT, "kt": KT, "vt": VT, "ft": FT,
            "wqt": np.ascontiguousarray(Wq[ds].T).astype(bf),
            "wkt": np.ascontiguousarray(Wk[ds].T).astype(bf),
            "wvt": np.ascontiguousarray(Wv[ds].T).astype(bf),
            "wot": np.ascontiguousarray(Wo[:, ds].T).astype(bf),
            "ubias": ((bq[ds, None] + urep) / 8.0).astype(np.float32),
            "vbias": ((bq[ds, None] + vrep) / 8.0).astype(np.float32),
            "kbias": bk[ds, None].astype(np.float32),
            "ones2": np.stack([np.ones(DK, np.float32),
                               np.zeros(DK, np.float32)]),
        })

    global _LAST_IN_MAPS
    _LAST_IN_MAPS = in_maps
    nc = _get_module()
    res = run_bass_kernel_spmd(nc, in_maps, core_ids=list(range(NCORES)))
    acc = np.zeros((D, L), np.float64)
    for r in res.results:
        acc += r["opart"].astype(np.float64)
    out = acc.T.astype(np.float32) + bo[None, :] + (bv @ Wo.T)[None, :]
    return out[None, :, :]
